# revision 1
# baseline (speedup 1.0000x reference)
"""Trainium2 Bass kernel: dense soft-MoE (router MLP + 8 expert MLPs + gated combine).

Problem shapes (hardcoded):
    x:   [16384, 512]   tokens
    experts (E=8): 512 -> 1024 -> 1024 -> 256, relu between, biases
    router: 512 -> 256 -> 256 -> 8, relu, softmax gates
    out: [16384, 256] = sum_e gates[:, e] * expert_e(x)

Sharding: data-parallel over the token axis — each of the 8 NeuronCores
processes 2048 tokens with a replicated copy of all weights. No collectives;
the host concatenates the 8 per-core outputs.

On-chip layout: activations are kept feature-major ([feature, token]) so every
layer's matmul contracts over the partition axis with the weight tile as the
stationary operand. The last expert layer switches to token-major so the
per-token gate becomes a per-partition scalar for the combine. Matmuls run as
float32r (full-rate fp32 mode, ~12-bit effective mantissa, 1 cycle/row at
free-dim >= 256); fp32r operands must be produced either by a DMA into an
fp32r-typed tile or by a compute op writing an fp32r tile. Free-dim biases are
folded into the PSUM accumulation via K=1 matmuls against a ones-row;
partition-dim biases ride the relu activation's bias operand.
"""

import sys

if "/opt/trn_rl_repo" not in sys.path:
    sys.path.insert(0, "/opt/trn_rl_repo")

from contextlib import nullcontext

import numpy as np

import concourse.mybir as mybir
import concourse.tile as tile
from concourse import bacc, bass_utils

N_CORES = 8
N_TOKENS = 16384
T = N_TOKENS // N_CORES  # 2048 tokens per core
D, W, O, E, R = 512, 1024, 256, 8, 256
NC = 512  # token chunk processed through one expert layer stack
P = 128
N_CHUNKS = T // NC  # 4
TT_PER_CHUNK = NC // P  # 4
N_TT = T // P  # 16 token tiles per core

F32 = mybir.dt.float32
F32R = mybir.dt.float32r
AF = mybir.ActivationFunctionType
ALU = mybir.AluOpType
AX = mybir.AxisListType


def _build(bench_iters=None):
    nc = bacc.Bacc("TRN2", target_bir_lowering=False)

    xT = nc.dram_tensor("xT", [D, T], F32, kind="ExternalInput")
    w1 = nc.dram_tensor("w1", [E, D, W], F32, kind="ExternalInput")
    b1 = nc.dram_tensor("b1", [E, W], F32, kind="ExternalInput")
    w2 = nc.dram_tensor("w2", [E, W, W], F32, kind="ExternalInput")
    b2 = nc.dram_tensor("b2", [E, W], F32, kind="ExternalInput")
    wout = nc.dram_tensor("wout", [E, W, O], F32, kind="ExternalInput")
    bout = nc.dram_tensor("bout", [E, O], F32, kind="ExternalInput")
    r1 = nc.dram_tensor("r1", [D, R], F32, kind="ExternalInput")
    rb1 = nc.dram_tensor("rb1", [R], F32, kind="ExternalInput")
    r2 = nc.dram_tensor("r2", [R, R], F32, kind="ExternalInput")
    rb2 = nc.dram_tensor("rb2", [R], F32, kind="ExternalInput")
    rout = nc.dram_tensor("rout", [R, E], F32, kind="ExternalInput")
    rbout = nc.dram_tensor("rbout", [E], F32, kind="ExternalInput")
    y = nc.dram_tensor("y", [T, O], F32, kind="ExternalOutput")

    with tile.TileContext(nc) as tc:
        with (
            tc.tile_pool(name="constp", bufs=1) as constp,
            tc.tile_pool(name="persist", bufs=1) as persist,
            tc.tile_pool(name="rw", bufs=1) as rwp,
            tc.tile_pool(name="smallp", bufs=4) as smallp,
            tc.tile_pool(name="xp", bufs=2) as xp,
            tc.tile_pool(name="ap", bufs=2) as ap,  # a1 / h1 / h2 share slots
            tc.tile_pool(name="a2p", bufs=1) as a2p,
            tc.tile_pool(name="wp", bufs=2) as wp,
            tc.tile_pool(name="w2p", bufs=3) as w2p,
            tc.tile_pool(name="psL", bufs=4, space="PSUM") as psL,
            tc.tile_pool(name="psS", bufs=2, space="PSUM") as psS,
            tc.tile_pool(name="psG", bufs=2, space="PSUM") as psG,
        ):
            # ---- one-time constants (outside any bench loop) ----
            ones = constp.tile([1, P], F32, name="ones")
            nc.vector.memset(ones[:], 1.0)
            boutsb = constp.tile([1, E, O], F32, name="boutsb")
            nc.sync.dma_start(boutsb[:], bout[:].rearrange("e o -> (e o)").unsqueeze(0))
            rboutsb = constp.tile([1, E], F32, name="rboutsb")
            nc.sync.dma_start(rboutsb[:], rbout[:].unsqueeze(0))
            r1sb = rwp.tile([P, D // P, R], F32R, name="r1sb")
            nc.sync.dma_start(
                r1sb[:], r1[:].rearrange("(ko p) r -> p ko r", p=P).bitcast(F32R)
            )
            r2sb = rwp.tile([P, R // P, R], F32R, name="r2sb")
            nc.sync.dma_start(
                r2sb[:], r2[:].rearrange("(ko p) r -> p ko r", p=P).bitcast(F32R)
            )
            routsb = rwp.tile([P, R // P, E], F32R, name="routsb")
            nc.sync.dma_start(
                routsb[:], rout[:].rearrange("(ko p) e -> p ko e", p=P).bitcast(F32R)
            )
            rb1sb = rwp.tile([P, R // P], F32, name="rb1sb")
            nc.sync.dma_start(rb1sb[:], rb1[:].rearrange("(fo p) -> p fo", p=P))
            rb2sb = rwp.tile([P, R // P], F32, name="rb2sb")
            nc.sync.dma_start(rb2sb[:], rb2[:].rearrange("(fo p) -> p fo", p=P))

            gates = persist.tile([P, N_TT, E], F32, name="gates")
            acc = persist.tile([P, N_TT, O], F32, name="acc")

            loop_cm = tc.For_i(0, bench_iters, 1) if bench_iters else nullcontext()
            with loop_cm:
                # ---------------- Router ----------------
                for ch in range(N_CHUNKS):
                    nsl = slice(ch * NC, (ch + 1) * NC)
                    xt = xp.tile([P, D // P, NC], F32R, name="xt")
                    nc.sync.dma_start(
                        xt[:],
                        xT[:, nsl].rearrange("(ko p) n -> p ko n", p=P).bitcast(F32R),
                    )
                    h1 = ap.tile([P, W // P, NC], F32R, name="act")[:, : R // P, :]
                    for fo in range(R // P):
                        ps = psL.tile([P, NC], F32, name="ps")
                        for ko in range(D // P):
                            nc.tensor.matmul(
                                ps[:],
                                r1sb[:, ko, fo * P : (fo + 1) * P],
                                xt[:, ko, :],
                                start=(ko == 0),
                                stop=(ko == D // P - 1),
                            )
                        nc.scalar.activation(
                            h1[:, fo, :], ps[:], AF.Relu, bias=rb1sb[:, fo : fo + 1]
                        )
                    h2 = ap.tile([P, W // P, NC], F32R, name="act")[:, : R // P, :]
                    for fo in range(R // P):
                        ps = psL.tile([P, NC], F32, name="ps")
                        for ko in range(R // P):
                            nc.tensor.matmul(
                                ps[:],
                                r2sb[:, ko, fo * P : (fo + 1) * P],
                                h1[:, ko, :],
                                start=(ko == 0),
                                stop=(ko == R // P - 1),
                            )
                        nc.scalar.activation(
                            h2[:, fo, :], ps[:], AF.Relu, bias=rb2sb[:, fo : fo + 1]
                        )
                    # logits + softmax, token-major [128 tokens, 8 experts]
                    for tt in range(TT_PER_CHUNK):
                        gt = ch * TT_PER_CHUNK + tt
                        tsl = slice(tt * P, (tt + 1) * P)
                        ps8 = psG.tile([P, E], F32, name="ps8")
                        for ko in range(R // P):
                            nc.tensor.matmul(
                                ps8[:],
                                h2[:, ko, tsl],
                                routsb[:, ko, :],
                                start=(ko == 0),
                                stop=False,
                            )
                        nc.tensor.matmul(
                            ps8[:], ones[:1, :], rboutsb[:1, :], start=False, stop=True
                        )
                        mx = smallp.tile([P, 1], F32, name="mx")
                        nc.vector.reduce_max(mx[:], ps8[:], axis=AX.X, negate=True)
                        eg = smallp.tile([P, E], F32, name="eg")
                        nc.scalar.activation(eg[:], ps8[:], AF.Exp, bias=mx[:])
                        sm = smallp.tile([P, 1], F32, name="sm")
                        nc.vector.reduce_sum(sm[:], eg[:], axis=AX.X)
                        rs = smallp.tile([P, 1], F32, name="rs")
                        nc.vector.reciprocal(rs[:], sm[:])
                        nc.vector.tensor_scalar_mul(gates[:, gt, :], eg[:], rs[:])

                # ---------------- Experts ----------------
                for e in range(E):
                    w1t = wp.tile([P, D // P, W], F32R, name="w1t")
                    nc.sync.dma_start(
                        w1t[:],
                        w1[e].rearrange("(ko p) f -> p ko f", p=P).bitcast(F32R),
                    )
                    w2h = []
                    for half in range(2):
                        w2t = w2p.tile([P, 4, W], F32R, name="w2h")
                        nc.sync.dma_start(
                            w2t[:],
                            w2[e, half * 512 : (half + 1) * 512]
                            .rearrange("(ko p) f -> p ko f", p=P)
                            .bitcast(F32R),
                        )
                        w2h.append(w2t)
                    wot = wp.tile([P, W // P, O], F32R, name="wot")
                    nc.sync.dma_start(
                        wot[:],
                        wout[e].rearrange("(ko p) o -> p ko o", p=P).bitcast(F32R),
                    )
                    b1t = wp.tile([P, W // P], F32, name="b1t")
                    nc.sync.dma_start(b1t[:], b1[e].rearrange("(fo p) -> p fo", p=P))
                    b2t = wp.tile([P, W // P], F32, name="b2t")
                    nc.sync.dma_start(b2t[:], b2[e].rearrange("(fo p) -> p fo", p=P))

                    for ch in range(N_CHUNKS):
                        nsl = slice(ch * NC, (ch + 1) * NC)
                        xt = xp.tile([P, D // P, NC], F32R, name="xt")
                        nc.sync.dma_start(
                            xt[:],
                            xT[:, nsl]
                            .rearrange("(ko p) n -> p ko n", p=P)
                            .bitcast(F32R),
                        )
                        a1 = ap.tile([P, W // P, NC], F32R, name="act")
                        for fo in range(W // P):
                            ps = psL.tile([P, NC], F32, name="ps")
                            for ko in range(D // P):
                                nc.tensor.matmul(
                                    ps[:],
                                    w1t[:, ko, fo * P : (fo + 1) * P],
                                    xt[:, ko, :],
                                    start=(ko == 0),
                                    stop=(ko == D // P - 1),
                                )
                            nc.scalar.activation(
                                a1[:, fo, :], ps[:], AF.Relu, bias=b1t[:, fo : fo + 1]
                            )
                        a2 = a2p.tile([P, W // P, NC], F32R, name="a2")
                        for fo in range(W // P):
                            ps = psL.tile([P, NC], F32, name="ps")
                            for ko in range(W // P):
                                nc.tensor.matmul(
                                    ps[:],
                                    w2h[ko // 4][:, ko % 4, fo * P : (fo + 1) * P],
                                    a1[:, ko, :],
                                    start=(ko == 0),
                                    stop=(ko == W // P - 1),
                                )
                            nc.scalar.activation(
                                a2[:, fo, :], ps[:], AF.Relu, bias=b2t[:, fo : fo + 1]
                            )
                        # final layer token-major + gated combine
                        for tt in range(TT_PER_CHUNK):
                            gt = ch * TT_PER_CHUNK + tt
                            tsl = slice(tt * P, (tt + 1) * P)
                            pso = psS.tile([P, O], F32, name="pso")
                            for ko in range(W // P):
                                nc.tensor.matmul(
                                    pso[:],
                                    a2[:, ko, tsl],
                                    wot[:, ko, :],
                                    start=(ko == 0),
                                    stop=False,
                                )
                            nc.tensor.matmul(
                                pso[:],
                                ones[:1, :],
                                boutsb[:1, e, :],
                                start=False,
                                stop=True,
                            )
                            g = gates[:, gt, e : e + 1]
                            if e == 0:
                                nc.vector.tensor_scalar_mul(acc[:, gt, :], pso[:], g)
                            else:
                                nc.vector.scalar_tensor_tensor(
                                    acc[:, gt, :],
                                    pso[:],
                                    g,
                                    acc[:, gt, :],
                                    ALU.mult,
                                    ALU.add,
                                )

                nc.sync.dma_start(y[:].rearrange("(gt p) o -> p gt o", p=P), acc[:])

    nc.compile()
    return nc


_CACHED_NC = None


def _get_nc():
    global _CACHED_NC
    if _CACHED_NC is None:
        _CACHED_NC = _build()
    return _CACHED_NC


def make_in_maps(inputs):
    x = np.asarray(inputs["x"], dtype=np.float32)
    shared = {
        name: np.ascontiguousarray(np.asarray(inputs[name], dtype=np.float32))
        for name in (
            "w1", "b1", "w2", "b2", "wout", "bout",
            "r1", "rb1", "r2", "rb2", "rout", "rbout",
        )
    }
    in_maps = []
    for c in range(N_CORES):
        xs = x[c * T : (c + 1) * T]
        m = {"xT": np.ascontiguousarray(xs.T)}
        m.update(shared)
        in_maps.append(m)
    return in_maps


def kernel(**inputs):
    in_maps = make_in_maps(inputs)
    nc = _get_nc()
    res = bass_utils.run_bass_kernel_spmd(nc, in_maps, core_ids=list(range(N_CORES)))
    return np.concatenate([res.results[c]["y"] for c in range(N_CORES)], axis=0)



# revision 2
# speedup vs baseline: 20.8368x; 20.8368x over previous
"""Trainium2 Bass kernel: dense soft-MoE (router MLP + 8 expert MLPs + gated combine).

Problem shapes (hardcoded):
    x:   [16384, 512]   tokens
    experts (E=8): 512 -> 1024 -> 1024 -> 256, relu between, biases
    router: 512 -> 256 -> 256 -> 8, relu, softmax gates
    out: [16384, 256] = sum_e gates[:, e] * expert_e(x)

Sharding: data-parallel over the token axis — each of the 8 NeuronCores
processes 2048 tokens with a replicated copy of all weights. No collectives;
the host concatenates the 8 per-core outputs.

v2 layout: weights and activations are bf16 (same 1 row/cycle PE rate as
fp32r, half the DMA bytes and SBUF footprint, and FWL halves LDWEIGHTS).
x is DMA'd once per iteration into a resident SBUF tile instead of per
(expert, chunk). Expert weights are double-buffered at expert granularity so
expert e+1's DMA overlaps expert e's compute. The fp32 K=1 ones-row bias
matmuls of the output layer (quarter-rate fp32 mode, ~55us/iter) are replaced
by one fp32r PSUM init per token tile: acc[:, tt, :] starts at
sum_e gates[n,e]*bout[e,:], computed as a K=8 matmul of transposed gates
against bout, and every expert then accumulates gate-scaled PSUM via DVE.
Partition-dim biases (b1, b2) ride the relu activation's bias operand.
"""

import sys

if "/opt/trn_rl_repo" not in sys.path:
    sys.path.insert(0, "/opt/trn_rl_repo")

from contextlib import nullcontext

import numpy as np

import concourse.mybir as mybir
import concourse.tile as tile
from concourse import bacc, bass_utils

N_CORES = 8
N_TOKENS = 16384
T = N_TOKENS // N_CORES  # 2048 tokens per core
D, W, O, E, R = 512, 1024, 256, 8, 256
NC = 512  # token chunk processed through one expert layer stack
P = 128
N_CHUNKS = T // NC  # 4
TT_PER_CHUNK = NC // P  # 4
N_TT = T // P  # 16 token tiles per core

F32 = mybir.dt.float32
F32R = mybir.dt.float32r
BF16 = mybir.dt.bfloat16
AF = mybir.ActivationFunctionType
ALU = mybir.AluOpType
AX = mybir.AxisListType


def _build(bench_iters=None):
    nc = bacc.Bacc("TRN2", target_bir_lowering=False)

    xT = nc.dram_tensor("xT", [D, T], BF16, kind="ExternalInput")
    w1 = nc.dram_tensor("w1", [E, D, W], BF16, kind="ExternalInput")
    b1 = nc.dram_tensor("b1", [E, W], F32, kind="ExternalInput")
    w2 = nc.dram_tensor("w2", [E, W, W], BF16, kind="ExternalInput")
    b2 = nc.dram_tensor("b2", [E, W], F32, kind="ExternalInput")
    wout = nc.dram_tensor("wout", [E, W, O], BF16, kind="ExternalInput")
    bout = nc.dram_tensor("bout", [E, O], F32, kind="ExternalInput")
    r1 = nc.dram_tensor("r1", [D, R], BF16, kind="ExternalInput")
    rb1 = nc.dram_tensor("rb1", [R], F32, kind="ExternalInput")
    r2 = nc.dram_tensor("r2", [R, R], BF16, kind="ExternalInput")
    rb2 = nc.dram_tensor("rb2", [R], F32, kind="ExternalInput")
    rout = nc.dram_tensor("rout", [R, E], BF16, kind="ExternalInput")
    rbout = nc.dram_tensor("rbout", [E], F32, kind="ExternalInput")
    idn = nc.dram_tensor("idn", [P, P], F32, kind="ExternalInput")
    y = nc.dram_tensor("y", [T, O], F32, kind="ExternalOutput")

    with tile.TileContext(nc) as tc:
        with (
            tc.tile_pool(name="constp", bufs=1) as constp,
            tc.tile_pool(name="rw", bufs=1) as rwp,
            tc.tile_pool(name="gp", bufs=2) as gp,
            tc.tile_pool(name="accp", bufs=2) as accp,
            tc.tile_pool(name="smallp", bufs=4) as smallp,
            tc.tile_pool(name="xp", bufs=2) as xp,
            tc.tile_pool(name="ap", bufs=2) as ap,  # a1 / h1 / h2 share slots
            tc.tile_pool(name="a2p", bufs=2) as a2p,
            tc.tile_pool(name="w1p", bufs=2) as w1p,
            tc.tile_pool(name="w2p", bufs=2) as w2p,
            tc.tile_pool(name="wop", bufs=2) as wop,
            tc.tile_pool(name="bp", bufs=4) as bp,
            tc.tile_pool(name="psL", bufs=4, space="PSUM") as psL,
            tc.tile_pool(name="psS", bufs=2, space="PSUM") as psS,
            tc.tile_pool(name="psG", bufs=2, space="PSUM") as psG,
        ):
            # ---- one-time constants (outside any bench loop) ----
            ones = constp.tile([1, P], F32, name="ones")
            nc.vector.memset(ones[:], 1.0)
            rboutsb = constp.tile([1, E], F32, name="rboutsb")
            nc.sync.dma_start(rboutsb[:], rbout[:].unsqueeze(0))
            idnsb = constp.tile([P, P], F32R, name="idnsb")
            nc.sync.dma_start(idnsb[:], idn[:].bitcast(F32R))
            boutEO = constp.tile([E, O], F32R, name="boutEO")
            nc.sync.dma_start(boutEO[:], bout[:].bitcast(F32R))
            r1sb = rwp.tile([P, D // P, R], BF16, name="r1sb")
            nc.sync.dma_start(r1sb[:], r1[:].rearrange("(ko p) r -> p ko r", p=P))
            r2sb = rwp.tile([P, R // P, R], BF16, name="r2sb")
            nc.sync.dma_start(r2sb[:], r2[:].rearrange("(ko p) r -> p ko r", p=P))
            routsb = rwp.tile([P, R // P, E], BF16, name="routsb")
            nc.sync.dma_start(routsb[:], rout[:].rearrange("(ko p) e -> p ko e", p=P))
            rb1sb = rwp.tile([P, R // P], F32, name="rb1sb")
            nc.sync.dma_start(rb1sb[:], rb1[:].rearrange("(fo p) -> p fo", p=P))
            rb2sb = rwp.tile([P, R // P], F32, name="rb2sb")
            nc.sync.dma_start(rb2sb[:], rb2[:].rearrange("(fo p) -> p fo", p=P))

            loop_cm = tc.For_i(0, bench_iters, 1) if bench_iters else nullcontext()
            with loop_cm:
                gates = gp.tile([P, N_TT, E], F32, name="gates")
                acc = accp.tile([P, N_TT, O], F32, name="acc")
                xsb = xp.tile([P, D // P, T], BF16, name="xsb")
                nc.sync.dma_start(
                    xsb[:], xT[:].rearrange("(ko p) n -> p ko n", p=P)
                )

                # ---------------- Router ----------------
                for ch in range(N_CHUNKS):
                    nsl = slice(ch * NC, (ch + 1) * NC)
                    h1 = ap.tile([P, W // P, NC], BF16, name="act")[:, : R // P, :]
                    for fo in range(R // P):
                        ps = psL.tile([P, NC], F32, name="ps")
                        for ko in range(D // P):
                            nc.tensor.matmul(
                                ps[:],
                                r1sb[:, ko, fo * P : (fo + 1) * P],
                                xsb[:, ko, nsl],
                                start=(ko == 0),
                                stop=(ko == D // P - 1),
                            )
                        nc.scalar.activation(
                            h1[:, fo, :], ps[:], AF.Relu, bias=rb1sb[:, fo : fo + 1]
                        )
                    h2 = ap.tile([P, W // P, NC], BF16, name="act")[:, : R // P, :]
                    for fo in range(R // P):
                        ps = psL.tile([P, NC], F32, name="ps")
                        for ko in range(R // P):
                            nc.tensor.matmul(
                                ps[:],
                                r2sb[:, ko, fo * P : (fo + 1) * P],
                                h1[:, ko, :],
                                start=(ko == 0),
                                stop=(ko == R // P - 1),
                            )
                        nc.scalar.activation(
                            h2[:, fo, :], ps[:], AF.Relu, bias=rb2sb[:, fo : fo + 1]
                        )
                    # logits + softmax, token-major [128 tokens, 8 experts]
                    for tt in range(TT_PER_CHUNK):
                        gt = ch * TT_PER_CHUNK + tt
                        tsl = slice(tt * P, (tt + 1) * P)
                        ps8 = psG.tile([P, E], F32, name="ps8")
                        for ko in range(R // P):
                            nc.tensor.matmul(
                                ps8[:],
                                h2[:, ko, tsl],
                                routsb[:, ko, :],
                                start=(ko == 0),
                                stop=False,
                            )
                        nc.tensor.matmul(
                            ps8[:], ones[:1, :], rboutsb[:1, :], start=False, stop=True
                        )
                        mx = smallp.tile([P, 1], F32, name="mx")
                        nc.vector.reduce_max(mx[:], ps8[:], axis=AX.X, negate=True)
                        eg = smallp.tile([P, E], F32, name="eg")
                        nc.scalar.activation(eg[:], ps8[:], AF.Exp, bias=mx[:])
                        sm = smallp.tile([P, 1], F32, name="sm")
                        nc.vector.reduce_sum(sm[:], eg[:], axis=AX.X)
                        rs = smallp.tile([P, 1], F32, name="rs")
                        nc.vector.reciprocal(rs[:], sm[:])
                        nc.vector.tensor_scalar_mul(gates[:, gt, :], eg[:], rs[:])

                # ---- acc init: acc[:, tt, :] = sum_e gates[:, tt, e] * bout[e, :]
                # gates [P, N_TT*E=128] transposed via PE -> gT[(tt,e), token]
                psT = psL.tile([P, P], F32, name="ps")
                nc.tensor.transpose(
                    psT[:],
                    gates[:].rearrange("p t e -> p (t e)").bitcast(F32R),
                    idnsb[:],
                )
                gTs = gp.tile([P, P], F32R, name="gTs")
                nc.vector.tensor_copy(gTs[:], psT[:].bitcast(F32R))
                for gt in range(N_TT):
                    psB = psS.tile([P, O], F32, name="pso")
                    nc.tensor.matmul(
                        psB[:],
                        gTs[gt * E : (gt + 1) * E, :],
                        boutEO[:],
                        start=True,
                        stop=True,
                    )
                    nc.scalar.activation(acc[:, gt, :], psB[:], AF.Copy)

                # ---------------- Experts ----------------
                for e in range(E):
                    w1t = w1p.tile([P, D // P, W], BF16, name="w1t")
                    nc.sync.dma_start(
                        w1t[:], w1[e].rearrange("(ko p) f -> p ko f", p=P)
                    )
                    w2t = w2p.tile([P, W // P, W], BF16, name="w2t")
                    nc.sync.dma_start(
                        w2t[:], w2[e].rearrange("(ko p) f -> p ko f", p=P)
                    )
                    wot = wop.tile([P, W // P, O], BF16, name="wot")
                    nc.sync.dma_start(
                        wot[:], wout[e].rearrange("(ko p) o -> p ko o", p=P)
                    )
                    b1t = bp.tile([P, W // P], F32, name="b1t")
                    nc.sync.dma_start(b1t[:], b1[e].rearrange("(fo p) -> p fo", p=P))
                    b2t = bp.tile([P, W // P], F32, name="b2t")
                    nc.sync.dma_start(b2t[:], b2[e].rearrange("(fo p) -> p fo", p=P))

                    for ch in range(N_CHUNKS):
                        nsl = slice(ch * NC, (ch + 1) * NC)
                        a1 = ap.tile([P, W // P, NC], BF16, name="act")
                        for fo in range(W // P):
                            ps = psL.tile([P, NC], F32, name="ps")
                            for ko in range(D // P):
                                nc.tensor.matmul(
                                    ps[:],
                                    w1t[:, ko, fo * P : (fo + 1) * P],
                                    xsb[:, ko, nsl],
                                    start=(ko == 0),
                                    stop=(ko == D // P - 1),
                                )
                            nc.scalar.activation(
                                a1[:, fo, :], ps[:], AF.Relu, bias=b1t[:, fo : fo + 1]
                            )
                        a2 = a2p.tile([P, W // P, NC], BF16, name="a2")
                        for fo in range(W // P):
                            ps = psL.tile([P, NC], F32, name="ps")
                            for ko in range(W // P):
                                nc.tensor.matmul(
                                    ps[:],
                                    w2t[:, ko, fo * P : (fo + 1) * P],
                                    a1[:, ko, :],
                                    start=(ko == 0),
                                    stop=(ko == W // P - 1),
                                )
                            nc.scalar.activation(
                                a2[:, fo, :], ps[:], AF.Relu, bias=b2t[:, fo : fo + 1]
                            )
                        # final layer token-major + gated combine
                        for tt in range(TT_PER_CHUNK):
                            gt = ch * TT_PER_CHUNK + tt
                            tsl = slice(tt * P, (tt + 1) * P)
                            pso = psS.tile([P, O], F32, name="pso")
                            for ko in range(W // P):
                                nc.tensor.matmul(
                                    pso[:],
                                    a2[:, ko, tsl],
                                    wot[:, ko, :],
                                    start=(ko == 0),
                                    stop=(ko == W // P - 1),
                                )
                            g = gates[:, gt, e : e + 1]
                            nc.vector.scalar_tensor_tensor(
                                acc[:, gt, :],
                                pso[:],
                                g,
                                acc[:, gt, :],
                                ALU.mult,
                                ALU.add,
                            )

                nc.sync.dma_start(y[:].rearrange("(gt p) o -> p gt o", p=P), acc[:])

    nc.compile()
    return nc


_CACHED_NC = None


def _get_nc():
    global _CACHED_NC
    if _CACHED_NC is None:
        _CACHED_NC = _build()
    return _CACHED_NC


def make_in_maps(inputs):
    import ml_dtypes

    bf16 = ml_dtypes.bfloat16
    x = np.asarray(inputs["x"], dtype=np.float32)
    shared = {}
    for name in ("w1", "w2", "wout", "r1", "r2", "rout"):
        shared[name] = np.ascontiguousarray(
            np.asarray(inputs[name], dtype=np.float32).astype(bf16)
        )
    for name in ("b1", "b2", "bout", "rb1", "rb2", "rbout"):
        shared[name] = np.ascontiguousarray(np.asarray(inputs[name], dtype=np.float32))
    shared["idn"] = np.eye(P, dtype=np.float32)
    in_maps = []
    for c in range(N_CORES):
        xs = x[c * T : (c + 1) * T]
        m = {"xT": np.ascontiguousarray(xs.T.astype(bf16))}
        m.update(shared)
        in_maps.append(m)
    return in_maps


def kernel(**inputs):
    in_maps = make_in_maps(inputs)
    nc = _get_nc()
    res = bass_utils.run_bass_kernel_spmd(nc, in_maps, core_ids=list(range(N_CORES)))
    return np.concatenate([res.results[c]["y"] for c in range(N_CORES)], axis=0)


# revision 8
# speedup vs baseline: 26.2668x; 1.2606x over previous
"""Trainium2 Bass kernel: dense soft-MoE (router MLP + 8 expert MLPs + gated combine).

Problem shapes (hardcoded):
    x:   [16384, 512]   tokens
    experts (E=8): 512 -> 1024 -> 1024 -> 256, relu between, biases
    router: 512 -> 256 -> 256 -> 8, relu, softmax gates
    out: [16384, 256] = sum_e gates[:, e] * expert_e(x)

Sharding: data-parallel over the token axis — each of the 8 NeuronCores
processes 2048 tokens with a replicated copy of all weights. No collectives;
the host concatenates the 8 per-core outputs.

v2 layout: weights and activations are bf16 (same 1 row/cycle PE rate as
fp32r, half the DMA bytes and SBUF footprint, and FWL halves LDWEIGHTS).
x is DMA'd once per iteration into a resident SBUF tile instead of per
(expert, chunk). Expert weights are double-buffered at expert granularity so
expert e+1's DMA overlaps expert e's compute. The fp32 K=1 ones-row bias
matmuls of the output layer (quarter-rate fp32 mode, ~55us/iter) are replaced
by one fp32r PSUM init per token tile: acc[:, tt, :] starts at
sum_e gates[n,e]*bout[e,:], computed as a K=8 matmul of transposed gates
against bout, and every expert then accumulates gate-scaled PSUM via DVE.
Partition-dim biases (b1, b2) ride the relu activation's bias operand.
"""

import sys

if "/opt/trn_rl_repo" not in sys.path:
    sys.path.insert(0, "/opt/trn_rl_repo")

from contextlib import nullcontext

import numpy as np

import concourse.mybir as mybir
import concourse.tile as tile
from concourse import bacc, bass_utils

N_CORES = 8
N_TOKENS = 16384
T = N_TOKENS // N_CORES  # 2048 tokens per core
D, W, O, E, R = 512, 1024, 256, 8, 256
NC = 512  # token chunk processed through one expert layer stack
P = 128
N_CHUNKS = T // NC  # 4
TT_PER_CHUNK = NC // P  # 4
N_TT = T // P  # 16 token tiles per core

F32 = mybir.dt.float32
F32R = mybir.dt.float32r
BF16 = mybir.dt.bfloat16
AF = mybir.ActivationFunctionType
ALU = mybir.AluOpType
AX = mybir.AxisListType


def _build(bench_iters=None):
    nc = bacc.Bacc("TRN2", target_bir_lowering=False)

    xT = nc.dram_tensor("xT", [D, T], BF16, kind="ExternalInput")
    w1 = nc.dram_tensor("w1", [E, D, W], BF16, kind="ExternalInput")
    b1 = nc.dram_tensor("b1", [E, W], F32, kind="ExternalInput")
    w2 = nc.dram_tensor("w2", [E, W, W], BF16, kind="ExternalInput")
    b2 = nc.dram_tensor("b2", [E, W], F32, kind="ExternalInput")
    wout = nc.dram_tensor("wout", [E, W, O], BF16, kind="ExternalInput")
    bout = nc.dram_tensor("bout", [E, O], F32, kind="ExternalInput")
    r1 = nc.dram_tensor("r1", [D, R], BF16, kind="ExternalInput")
    rb1 = nc.dram_tensor("rb1", [R], F32, kind="ExternalInput")
    r2 = nc.dram_tensor("r2", [R, R], BF16, kind="ExternalInput")
    rb2 = nc.dram_tensor("rb2", [R], F32, kind="ExternalInput")
    rout = nc.dram_tensor("rout", [R, E], BF16, kind="ExternalInput")
    rbout = nc.dram_tensor("rbout", [E], F32, kind="ExternalInput")
    idn = nc.dram_tensor("idn", [P, P], F32, kind="ExternalInput")
    y = nc.dram_tensor("y", [T, O], F32, kind="ExternalOutput")

    with tile.TileContext(nc) as tc:
        with (
            tc.tile_pool(name="constp", bufs=1) as constp,
            tc.tile_pool(name="rw", bufs=1) as rwp,
            tc.tile_pool(name="gp", bufs=2) as gp,
            tc.tile_pool(name="accp", bufs=2) as accp,
            tc.tile_pool(name="smallp", bufs=4) as smallp,
            tc.tile_pool(name="xp", bufs=2) as xp,
            tc.tile_pool(name="ap", bufs=2) as ap,  # a1 / h1 / h2 share slots
            tc.tile_pool(name="a2p", bufs=2) as a2p,
            tc.tile_pool(name="w1p", bufs=2) as w1p,
            tc.tile_pool(name="w2p", bufs=2) as w2p,
            tc.tile_pool(name="wop", bufs=2) as wop,
            tc.tile_pool(name="bp", bufs=4) as bp,
            tc.tile_pool(name="psL", bufs=4, space="PSUM") as psL,
            tc.tile_pool(name="psS", bufs=2, space="PSUM") as psS,
            tc.tile_pool(name="psG", bufs=2, space="PSUM") as psG,
        ):
            # ---- one-time constants (outside any bench loop) ----
            ones = constp.tile([1, P], F32, name="ones")
            nc.vector.memset(ones[:], 1.0)
            rboutsb = constp.tile([1, E], F32, name="rboutsb")
            nc.sync.dma_start(rboutsb[:], rbout[:].unsqueeze(0))
            idnsb = constp.tile([P, P], F32R, name="idnsb")
            nc.sync.dma_start(idnsb[:], idn[:].bitcast(F32R))
            boutEO = constp.tile([E, O], F32R, name="boutEO")
            nc.sync.dma_start(boutEO[:], bout[:].bitcast(F32R))
            r1sb = rwp.tile([P, D // P, R], BF16, name="r1sb")
            nc.sync.dma_start(r1sb[:], r1[:].rearrange("(ko p) r -> p ko r", p=P))
            r2sb = rwp.tile([P, R // P, R], BF16, name="r2sb")
            nc.sync.dma_start(r2sb[:], r2[:].rearrange("(ko p) r -> p ko r", p=P))
            routsb = rwp.tile([P, R // P, E], BF16, name="routsb")
            nc.sync.dma_start(routsb[:], rout[:].rearrange("(ko p) e -> p ko e", p=P))
            rb1sb = rwp.tile([P, R // P], F32, name="rb1sb")
            nc.sync.dma_start(rb1sb[:], rb1[:].rearrange("(fo p) -> p fo", p=P))
            rb2sb = rwp.tile([P, R // P], F32, name="rb2sb")
            nc.sync.dma_start(rb2sb[:], rb2[:].rearrange("(fo p) -> p fo", p=P))

            loop_cm = tc.For_i(0, bench_iters, 1) if bench_iters else nullcontext()
            with loop_cm:
                gates = gp.tile([P, N_TT, E], F32R, name="gates")
                gT8 = gp.tile([E, T], F32R, name="gT8")
                acc = accp.tile([P, N_TT, O], F32, name="acc")
                xsb = xp.tile([P, D // P, T], BF16, name="xsb")
                nc.sync.dma_start(
                    xsb[:], xT[:].rearrange("(ko p) n -> p ko n", p=P)
                )

                # ---------------- Router ----------------
                for ch in range(N_CHUNKS):
                    nsl = slice(ch * NC, (ch + 1) * NC)
                    h1 = ap.tile([P, W // P, NC], BF16, name="act")[:, : R // P, :]
                    for fo in range(R // P):
                        ps = psL.tile([P, NC], F32, name="ps")
                        for ko in range(D // P):
                            nc.tensor.matmul(
                                ps[:],
                                r1sb[:, ko, fo * P : (fo + 1) * P],
                                xsb[:, ko, nsl],
                                start=(ko == 0),
                                stop=(ko == D // P - 1),
                            )
                        nc.scalar.activation(
                            h1[:, fo, :], ps[:], AF.Relu, bias=rb1sb[:, fo : fo + 1]
                        )
                    h2 = ap.tile([P, W // P, NC], BF16, name="act")[:, : R // P, :]
                    for fo in range(R // P):
                        ps = psL.tile([P, NC], F32, name="ps")
                        for ko in range(R // P):
                            nc.tensor.matmul(
                                ps[:],
                                r2sb[:, ko, fo * P : (fo + 1) * P],
                                h1[:, ko, :],
                                start=(ko == 0),
                                stop=(ko == R // P - 1),
                            )
                        nc.scalar.activation(
                            h2[:, fo, :], ps[:], AF.Relu, bias=rb2sb[:, fo : fo + 1]
                        )
                    # logits + softmax, token-major [128 tokens, 8 experts]
                    for tt in range(TT_PER_CHUNK):
                        gt = ch * TT_PER_CHUNK + tt
                        tsl = slice(tt * P, (tt + 1) * P)
                        ps8 = psG.tile([P, E], F32, name="ps8")
                        for ko in range(R // P):
                            nc.tensor.matmul(
                                ps8[:],
                                h2[:, ko, tsl],
                                routsb[:, ko, :],
                                start=(ko == 0),
                                stop=False,
                            )
                        nc.tensor.matmul(
                            ps8[:], ones[:1, :], rboutsb[:1, :], start=False, stop=True
                        )
                        mx = smallp.tile([P, 1], F32, name="mx")
                        nc.vector.reduce_max(mx[:], ps8[:], axis=AX.X, negate=True)
                        eg = smallp.tile([P, E], F32, name="eg")
                        nc.scalar.activation(eg[:], ps8[:], AF.Exp, bias=mx[:])
                        sm = smallp.tile([P, 1], F32, name="sm")
                        nc.vector.reduce_sum(sm[:], eg[:], axis=AX.X)
                        rs = smallp.tile([P, 1], F32, name="rs")
                        nc.vector.reciprocal(rs[:], sm[:])
                        nc.vector.tensor_scalar_mul(gates[:, gt, :], eg[:], rs[:])
                    # expert-major gate copy gT8[e, n] for this chunk via PE
                    # transpose (partition-base rule forbids slicing gates at
                    # 8-row offsets, so transpose per token tile into one
                    # [8, NC] PSUM bank, then copy to SBUF)
                    psT8 = psG.tile([E, NC], F32R, name="ps8")
                    for tt in range(TT_PER_CHUNK):
                        gt = ch * TT_PER_CHUNK + tt
                        nc.tensor.transpose(
                            psT8[:, tt * P : (tt + 1) * P],
                            gates[:, gt, :],
                            idnsb[:],
                        )
                    nc.vector.tensor_copy(gT8[:, nsl], psT8[:])

                # ---- acc init: acc[:, tt, :] = sum_e gates[:, tt, e] * bout[e, :]
                for gt in range(N_TT):
                    psB = psS.tile([P, O], F32, name="pso")
                    nc.tensor.matmul(
                        psB[:],
                        gT8[:, gt * P : (gt + 1) * P],
                        boutEO[:],
                        start=True,
                        stop=True,
                    )
                    nc.scalar.activation(acc[:, gt, :], psB[:], AF.Copy)

                # ---------------- Experts ----------------
                for e in range(E):
                    w1t = w1p.tile([P, D // P, W], BF16, name="w1t")
                    nc.sync.dma_start(
                        w1t[:], w1[e].rearrange("(ko p) f -> p ko f", p=P)
                    )
                    w2t = w2p.tile([P, W // P, W], BF16, name="w2t")
                    nc.sync.dma_start(
                        w2t[:], w2[e].rearrange("(ko p) f -> p ko f", p=P)
                    )
                    wot = wop.tile([P, W // P, O], BF16, name="wot")
                    nc.sync.dma_start(
                        wot[:], wout[e].rearrange("(ko p) o -> p ko o", p=P)
                    )
                    b1t = bp.tile([P, W // P], F32, name="b1t")
                    nc.sync.dma_start(b1t[:], b1[e].rearrange("(fo p) -> p fo", p=P))
                    b2t = bp.tile([P, W // P], F32, name="b2t")
                    nc.sync.dma_start(b2t[:], b2[e].rearrange("(fo p) -> p fo", p=P))

                    for ch in range(N_CHUNKS):
                        nsl = slice(ch * NC, (ch + 1) * NC)
                        a1 = ap.tile([P, W // P, NC], BF16, name="act")
                        for fo in range(W // P):
                            ps = psL.tile([P, NC], F32, name="ps")
                            for ko in range(D // P):
                                nc.tensor.matmul(
                                    ps[:],
                                    w1t[:, ko, fo * P : (fo + 1) * P],
                                    xsb[:, ko, nsl],
                                    start=(ko == 0),
                                    stop=(ko == D // P - 1),
                                )
                            nc.scalar.activation(
                                a1[:, fo, :], ps[:], AF.Relu, bias=b1t[:, fo : fo + 1]
                            )
                        a2 = a2p.tile([P, W // P, NC], BF16, name="a2")
                        for fo in range(W // P):
                            ps = psL.tile([P, NC], F32, name="ps")
                            for ko in range(W // P):
                                nc.tensor.matmul(
                                    ps[:],
                                    w2t[:, ko, fo * P : (fo + 1) * P],
                                    a1[:, ko, :],
                                    start=(ko == 0),
                                    stop=(ko == W // P - 1),
                                )
                            nc.scalar.activation(
                                a2[:, fo, :], ps[:], AF.Relu, bias=b2t[:, fo : fo + 1]
                            )
                        # final layer token-major + gated combine
                        for tt in range(TT_PER_CHUNK):
                            gt = ch * TT_PER_CHUNK + tt
                            tsl = slice(tt * P, (tt + 1) * P)
                            pso = psS.tile([P, O], F32, name="pso")
                            for ko in range(W // P):
                                nc.tensor.matmul(
                                    pso[:],
                                    a2[:, ko, tsl],
                                    wot[:, ko, :],
                                    start=(ko == 0),
                                    stop=(ko == W // P - 1),
                                )
                            g = gates[:, gt, e : e + 1].bitcast(F32)
                            nc.vector.scalar_tensor_tensor(
                                acc[:, gt, :],
                                pso[:],
                                g,
                                acc[:, gt, :],
                                ALU.mult,
                                ALU.add,
                            )

                nc.sync.dma_start(y[:].rearrange("(gt p) o -> p gt o", p=P), acc[:])

    nc.compile()
    return nc


_CACHED_NC = None


def _get_nc():
    global _CACHED_NC
    if _CACHED_NC is None:
        _CACHED_NC = _build()
    return _CACHED_NC


def make_in_maps(inputs):
    import ml_dtypes

    bf16 = ml_dtypes.bfloat16
    x = np.asarray(inputs["x"], dtype=np.float32)
    shared = {}
    for name in ("w1", "w2", "wout", "r1", "r2", "rout"):
        shared[name] = np.ascontiguousarray(
            np.asarray(inputs[name], dtype=np.float32).astype(bf16)
        )
    for name in ("b1", "b2", "bout", "rb1", "rb2", "rbout"):
        shared[name] = np.ascontiguousarray(np.asarray(inputs[name], dtype=np.float32))
    shared["idn"] = np.eye(P, dtype=np.float32)
    in_maps = []
    for c in range(N_CORES):
        xs = x[c * T : (c + 1) * T]
        m = {"xT": np.ascontiguousarray(xs.T.astype(bf16))}
        m.update(shared)
        in_maps.append(m)
    return in_maps


def kernel(**inputs):
    in_maps = make_in_maps(inputs)
    nc = _get_nc()
    res = bass_utils.run_bass_kernel_spmd(nc, in_maps, core_ids=list(range(N_CORES)))
    return np.concatenate([res.results[c]["y"] for c in range(N_CORES)], axis=0)
